# revision 11
# baseline (speedup 1.0000x reference)
"""Trainium2 Bass kernel for the DFAProb problem.

Computes, per batch row (8 DFA states):
    unnorm[j] = s0[(j-1)%8]*a*b + s0[(j-2)%8]*a*(1-b) + s0[j]*(1-a)
    denom     = sum_j unnorm[j]        (== sum_j s0[j], since guards sum to 1
                                        and the rolls are permutations)
    next      = clip(unnorm/denom, EPS, 1-EPS)
    log_next  = log(next)
    accepted  = next[7]

Full inputs (B=2,000,000 rows) are sharded row-wise across 8 NeuronCores;
each core runs an identical Bass/Tile program on its 250,000-row shard.
"""

import sys

import numpy as np

for _p in ("/opt/trn_rl_repo",):
    if _p not in sys.path:
        sys.path.insert(0, _p)

import concourse.bacc as bacc
import concourse.bass as bass
import concourse.mybir as mybir
from concourse.bass_utils import run_bass_kernel_spmd
from concourse.tile import TileContext

F32 = mybir.dt.float32
ALU = mybir.AluOpType
ACTF = mybir.ActivationFunctionType

EPS = np.float32(1e-7)
ONE_MINUS_EPS = np.float32(1.0 - 1e-7)

N_CORES = 8
B_FULL = 2_000_000
N_STATES = 8


def _bcast(tile_ap: bass.AP, k: int) -> bass.AP:
    """View a [P, R] AP as [P, R, k] by appending a stride-0 dim."""
    return bass.AP(tile_ap.tensor, tile_ap.offset, [*tile_ap.ap, [0, k]])


def build_program(n_rows: int, P: int, R: int, big_bufs: int = 4,
                  small_bufs: int = 6, skew: int = 2) -> bass.Bass:
    """Build the per-core Bass program.

    n_rows: rows in this core's shard; must equal P * RPP with RPP % R == 0.
    P: partitions used (<=128). R: rows per partition per chunk.
    """
    RPP = n_rows // P
    assert P * RPP == n_rows and RPP % R == 0
    n_iters = RPP // R

    nc = bacc.Bacc()
    s0 = nc.dram_tensor("s0", [n_rows, N_STATES], F32, kind="ExternalInput")
    a = nc.dram_tensor("a", [n_rows], F32, kind="ExternalInput")
    b = nc.dram_tensor("b", [n_rows], F32, kind="ExternalInput")
    nxt = nc.dram_tensor("next_state", [n_rows, N_STATES], F32, kind="ExternalOutput")
    lg = nc.dram_tensor("log_next_state", [n_rows, N_STATES], F32,
                        kind="ExternalOutput")
    acc = nc.dram_tensor("accepted", [n_rows], F32, kind="ExternalOutput")

    s0v = s0[:, :].rearrange("(p q) j -> p q j", p=P)      # [P, RPP, 8]
    nxtv = nxt[:, :].rearrange("(p q) j -> p q j", p=P)
    lgv = lg[:, :].rearrange("(p q) j -> p q j", p=P)
    av = a[:].rearrange("(p q) -> p q", p=P)               # [P, RPP]
    bv = b[:].rearrange("(p q) -> p q", p=P)
    accv = acc[:].rearrange("(p q) -> p q", p=P)

    with TileContext(nc) as tc:
        with tc.tile_pool(name="big", bufs=big_bufs) as bigp, \
             tc.tile_pool(name="small", bufs=small_bufs) as smallp:
            live: dict = {}

            def produce(i):
                sl = slice(i * R, (i + 1) * R)
                s0_t = bigp.tile([P, R, N_STATES], F32, tag="s0")
                nc.sync.dma_start(out=s0_t[:], in_=s0v[:, sl, :])
                a_t = smallp.tile([P, R], F32, tag="a")
                nc.sync.dma_start(out=a_t[:], in_=av[:, sl])
                b_t = smallp.tile([P, R], F32, tag="b")
                nc.sync.dma_start(out=b_t[:], in_=bv[:, sl])

                # per-row guard scalars, scaled by 1/sum(s0)
                S = smallp.tile([P, R], F32, tag="S")
                nc.vector.tensor_reduce(S[:], s0_t[:], mybir.AxisListType.X,
                                        ALU.add)
                rcp = smallp.tile([P, R], F32, tag="rcp")
                nc.vector.reciprocal(rcp[:], S[:])

                ab = smallp.tile([P, R], F32, tag="ab")
                nc.vector.tensor_mul(ab[:], a_t[:], b_t[:])
                abp = smallp.tile([P, R], F32, tag="abp")
                nc.vector.tensor_mul(abp[:], ab[:], rcp[:])

                na = smallp.tile([P, R], F32, tag="na")      # 1 - a
                nc.scalar.activation(na[:], a_t[:], ACTF.Identity,
                                     bias=1.0, scale=-1.0)

                anb = smallp.tile([P, R], F32, tag="anb")    # a*(1-b) = a - ab
                nc.gpsimd.tensor_sub(anb[:], a_t[:], ab[:])
                anbp = smallp.tile([P, R], F32, tag="anbp")
                nc.gpsimd.tensor_mul(anbp[:], anb[:], rcp[:])
                nap = smallp.tile([P, R], F32, tag="nap")
                nc.gpsimd.tensor_mul(nap[:], na[:], rcp[:])

                # unnorm/denom = roll1*ab' + roll2*anb' + s0*na'
                t1 = bigp.tile([P, R, N_STATES], F32, tag="t1")
                nc.vector.tensor_tensor(t1[:, :, 1:8], s0_t[:, :, 0:7],
                                        _bcast(abp[:], 7), ALU.mult)
                nc.vector.tensor_tensor(t1[:, :, 0:1], s0_t[:, :, 7:8],
                                        _bcast(abp[:], 1), ALU.mult)

                t2 = bigp.tile([P, R, N_STATES], F32, tag="t2")
                nc.vector.tensor_tensor(t2[:, :, 2:8], s0_t[:, :, 0:6],
                                        _bcast(anbp[:], 6), ALU.mult)
                nc.vector.tensor_tensor(t2[:, :, 0:2], s0_t[:, :, 6:8],
                                        _bcast(anbp[:], 2), ALU.mult)

                t3 = bigp.tile([P, R, N_STATES], F32, tag="t3")
                nc.gpsimd.tensor_tensor(t3[:], s0_t[:], _bcast(nap[:], 8),
                                        ALU.mult)

                nc.vector.tensor_add(t1[:], t1[:], t2[:])    # t1 = t1 + t2
                live[i] = (t1, t2, t3)

            def consume(i):
                sl = slice(i * R, (i + 1) * R)
                t1, t2, t3 = live.pop(i)
                nc.gpsimd.tensor_add(t3[:], t1[:], t3[:])    # t3 = u

                c = bigp.tile([P, R, N_STATES], F32, tag="c")
                nc.vector.tensor_scalar(c[:], t3[:], float(EPS),
                                        float(ONE_MINUS_EPS), ALU.max, ALU.min)

                l = t2                                       # reuse t2's slot
                nc.scalar.activation(l[:], c[:], ACTF.Ln)

                at = smallp.tile([P, R], F32, tag="at")
                nc.scalar.copy(at[:], c[:, :, 7])

                nc.scalar.dma_start(out=nxtv[:, sl, :], in_=c[:])
                nc.scalar.dma_start(out=lgv[:, sl, :], in_=l[:])
                nc.scalar.dma_start(out=accv[:, sl], in_=at[:])

            for i in range(n_iters + skew):
                if i < n_iters:
                    produce(i)
                if i >= skew:
                    consume(i - skew)

    nc.compile()
    return nc


_CACHE: dict = {}


def _get_program() -> bass.Bass:
    if "nc" not in _CACHE:
        _CACHE["nc"] = build_program(B_FULL // N_CORES, P=125, R=200,
                                     big_bufs=4, small_bufs=6, skew=2)
    return _CACHE["nc"]


def kernel(log_s0, s0, a, b, _trace: bool = False):
    s0 = np.ascontiguousarray(np.asarray(s0, dtype=np.float32))
    a = np.ascontiguousarray(np.asarray(a, dtype=np.float32))
    b = np.ascontiguousarray(np.asarray(b, dtype=np.float32))
    assert s0.shape == (B_FULL, N_STATES)

    n = B_FULL // N_CORES
    in_maps = [
        {"s0": s0[c * n:(c + 1) * n], "a": a[c * n:(c + 1) * n],
         "b": b[c * n:(c + 1) * n]}
        for c in range(N_CORES)
    ]
    nc = _get_program()
    res = run_bass_kernel_spmd(nc, in_maps, core_ids=list(range(N_CORES)),
                               trace=_trace)
    if _trace:
        _CACHE["last_results"] = res
    log_next = np.concatenate([r["log_next_state"] for r in res.results])
    next_state = np.concatenate([r["next_state"] for r in res.results])
    accepted = np.concatenate([r["accepted"] for r in res.results])
    return log_next, next_state, accepted


# revision 20
# speedup vs baseline: 30.1853x; 30.1853x over previous
"""Trainium2 Bass kernel for the DFAProb problem.

Computes, per batch row (8 DFA states):
    unnorm[j] = s0[(j-1)%8]*a*b + s0[(j-2)%8]*a*(1-b) + s0[j]*(1-a)
    denom     = sum_j unnorm[j]        (== sum_j s0[j], since guards sum to 1
                                        and the rolls are permutations)
    next      = clip(unnorm/denom, EPS, 1-EPS)
    log_next  = log(next)
    accepted  = next[7]

Full inputs (B=2,000,000 rows) are sharded row-wise across 8 NeuronCores;
each core runs an identical Bass/Tile program on its 250,000-row shard.
"""

import sys

import numpy as np

for _p in ("/opt/trn_rl_repo",):
    if _p not in sys.path:
        sys.path.insert(0, _p)

import concourse.bacc as bacc
import concourse.bass as bass
import concourse.mybir as mybir
from concourse.bass_utils import run_bass_kernel_spmd
from concourse.tile import TileContext

F32 = mybir.dt.float32
ALU = mybir.AluOpType
ACTF = mybir.ActivationFunctionType

EPS = np.float32(1e-7)
ONE_MINUS_EPS = np.float32(1.0 - 1e-7)

N_CORES = 8
B_FULL = 2_000_000
N_STATES = 8


def _bcast(tile_ap: bass.AP, k: int) -> bass.AP:
    """View a [P, R] AP as [P, R, k] by appending a stride-0 dim."""
    return bass.AP(tile_ap.tensor, tile_ap.offset, [*tile_ap.ap, [0, k]])


def build_program(n_rows: int, P: int, R: int, big_bufs: int = 4,
                  small_bufs: int = 6, skew: int = 2,
                  reps: int = 1, swap_adds: bool = False,
                  mode: str = "full") -> bass.Bass:
    """Build the per-core Bass program.

    n_rows: rows in this core's shard; must equal P * RPP with RPP % R == 0.
    P: partitions used (<=128). R: rows per partition per chunk.
    """
    RPP = n_rows // P
    assert P * RPP == n_rows and RPP % R == 0
    n_iters = RPP // R

    nc = bacc.Bacc()
    s0 = nc.dram_tensor("s0", [n_rows, N_STATES], F32, kind="ExternalInput")
    a = nc.dram_tensor("a", [n_rows], F32, kind="ExternalInput")
    b = nc.dram_tensor("b", [n_rows], F32, kind="ExternalInput")
    nxt = nc.dram_tensor("next_state", [n_rows, N_STATES], F32, kind="ExternalOutput")
    lg = nc.dram_tensor("log_next_state", [n_rows, N_STATES], F32,
                        kind="ExternalOutput")
    acc = nc.dram_tensor("accepted", [n_rows], F32, kind="ExternalOutput")

    s0v = s0[:, :].rearrange("(p q) j -> p q j", p=P)      # [P, RPP, 8]
    nxtv = nxt[:, :].rearrange("(p q) j -> p q j", p=P)
    lgv = lg[:, :].rearrange("(p q) j -> p q j", p=P)
    av = a[:].rearrange("(p q) -> p q", p=P)               # [P, RPP]
    bv = b[:].rearrange("(p q) -> p q", p=P)
    accv = acc[:].rearrange("(p q) -> p q", p=P)

    with TileContext(nc) as tc:
        with tc.tile_pool(name="big", bufs=big_bufs) as bigp, \
             tc.tile_pool(name="small", bufs=small_bufs) as smallp:
            live: dict = {}

            def produce(i):
                sl = slice(i * R, (i + 1) * R)
                s0_t = bigp.tile([P, R, N_STATES], F32, tag="s0")
                nc.sync.dma_start(out=s0_t[:], in_=s0v[:, sl, :])
                a_t = smallp.tile([P, R], F32, tag="a")
                nc.sync.dma_start(out=a_t[:], in_=av[:, sl])
                b_t = smallp.tile([P, R], F32, tag="b")
                nc.sync.dma_start(out=b_t[:], in_=bv[:, sl])
                if mode == "dma":
                    live[i] = (s0_t, a_t)
                    return

                # per-row guard scalars, scaled by 1/sum(s0)
                S = smallp.tile([P, R], F32, tag="S")
                nc.vector.tensor_reduce(S[:], s0_t[:], mybir.AxisListType.X,
                                        ALU.add)
                rcp = smallp.tile([P, R], F32, tag="rcp")
                nc.vector.reciprocal(rcp[:], S[:])

                ab = smallp.tile([P, R], F32, tag="ab")
                nc.vector.tensor_mul(ab[:], a_t[:], b_t[:])
                abp = smallp.tile([P, R], F32, tag="abp")
                nc.vector.tensor_mul(abp[:], ab[:], rcp[:])

                na = smallp.tile([P, R], F32, tag="na")      # 1 - a
                nc.scalar.activation(na[:], a_t[:], ACTF.Identity,
                                     bias=1.0, scale=-1.0)

                anb = smallp.tile([P, R], F32, tag="anb")    # a*(1-b) = a - ab
                nc.gpsimd.tensor_sub(anb[:], a_t[:], ab[:])
                anbp = smallp.tile([P, R], F32, tag="anbp")
                nc.gpsimd.tensor_mul(anbp[:], anb[:], rcp[:])
                nap = smallp.tile([P, R], F32, tag="nap")
                nc.gpsimd.tensor_mul(nap[:], na[:], rcp[:])

                # unnorm/denom = roll1*ab' + roll2*anb' + s0*na'
                t1 = bigp.tile([P, R, N_STATES], F32, tag="t1")
                nc.vector.tensor_tensor(t1[:, :, 1:8], s0_t[:, :, 0:7],
                                        _bcast(abp[:], 7), ALU.mult)
                nc.vector.tensor_tensor(t1[:, :, 0:1], s0_t[:, :, 7:8],
                                        _bcast(abp[:], 1), ALU.mult)

                t2 = bigp.tile([P, R, N_STATES], F32, tag="t2")
                nc.vector.tensor_tensor(t2[:, :, 2:8], s0_t[:, :, 0:6],
                                        _bcast(anbp[:], 6), ALU.mult)
                nc.vector.tensor_tensor(t2[:, :, 0:2], s0_t[:, :, 6:8],
                                        _bcast(anbp[:], 2), ALU.mult)

                t3 = bigp.tile([P, R, N_STATES], F32, tag="t3")
                if swap_adds:
                    nc.vector.tensor_tensor(t3[:], s0_t[:], _bcast(nap[:], 8),
                                            ALU.mult)
                    nc.gpsimd.tensor_add(t1[:], t1[:], t2[:])
                else:
                    nc.gpsimd.tensor_tensor(t3[:], s0_t[:], _bcast(nap[:], 8),
                                            ALU.mult)
                    nc.vector.tensor_add(t1[:], t1[:], t2[:])
                live[i] = (t1, t2, t3)

            def consume(i):
                sl = slice(i * R, (i + 1) * R)
                if mode == "dma":
                    s0_t, a_t = live.pop(i)
                    nc.scalar.dma_start(out=nxtv[:, sl, :], in_=s0_t[:])
                    nc.scalar.dma_start(out=lgv[:, sl, :], in_=s0_t[:])
                    nc.scalar.dma_start(out=accv[:, sl], in_=a_t[:])
                    return
                t1, t2, t3 = live.pop(i)
                nc.gpsimd.tensor_add(t3[:], t1[:], t3[:])    # t3 = u

                c = t3                                       # clip in place
                nc.vector.tensor_scalar(c[:], t3[:], float(EPS),
                                        float(ONE_MINUS_EPS), ALU.max, ALU.min)

                l = t2                                       # reuse t2's slot
                nc.scalar.activation(l[:], c[:], ACTF.Ln)

                at = smallp.tile([P, R], F32, tag="at")
                nc.scalar.copy(at[:], c[:, :, 7])

                nc.scalar.dma_start(out=nxtv[:, sl, :], in_=c[:])
                nc.scalar.dma_start(out=lgv[:, sl, :], in_=l[:])
                nc.scalar.dma_start(out=accv[:, sl], in_=at[:])

            for _rep in range(reps):
                for i in range(n_iters + skew):
                    if i < n_iters:
                        produce(i)
                    if i >= skew:
                        consume(i - skew)

    nc.compile()
    return nc


_CACHE: dict = {}


def _get_program() -> bass.Bass:
    if "nc" not in _CACHE:
        _CACHE["nc"] = build_program(B_FULL // N_CORES, P=125, R=200,
                                     big_bufs=4, small_bufs=6, skew=2)
    return _CACHE["nc"]


def kernel(log_s0, s0, a, b, _trace: bool = False):
    s0 = np.ascontiguousarray(np.asarray(s0, dtype=np.float32))
    a = np.ascontiguousarray(np.asarray(a, dtype=np.float32))
    b = np.ascontiguousarray(np.asarray(b, dtype=np.float32))
    assert s0.shape == (B_FULL, N_STATES)

    n = B_FULL // N_CORES
    in_maps = [
        {"s0": s0[c * n:(c + 1) * n], "a": a[c * n:(c + 1) * n],
         "b": b[c * n:(c + 1) * n]}
        for c in range(N_CORES)
    ]
    nc = _get_program()
    res = run_bass_kernel_spmd(nc, in_maps, core_ids=list(range(N_CORES)),
                               trace=_trace)
    if _trace:
        _CACHE["last_results"] = res
    log_next = np.concatenate([r["log_next_state"] for r in res.results])
    next_state = np.concatenate([r["next_state"] for r in res.results])
    accepted = np.concatenate([r["accepted"] for r in res.results])
    return log_next, next_state, accepted
